# revision 41
# baseline (speedup 1.0000x reference)
"""Trainium2 Bass kernel for nn_NMSquaredGaussianMixture.

Math: output = -(log(sum_n g_n^2) - log z) / N
  g_n = sum_k c_k * exp(E_k(x_n)),  c_k = w_k / sqrt((2pi)^2 det S_k)
  E_k(x) = -0.5 (x-mu_k)^T S_k^{-1} (x-mu_k)
  z     = sum_ij w_i w_j N(mu_i - mu_j; 0, S_i + S_j)   (tiny, host-side)

Device pipeline (per core, data-parallel over samples), per super-tile of
8192 samples (16 groups x 512):
  E = W1 @ F     features F = [y0^2, y0*y1, y1^2, y0, y1, 1], y = x - ctr
                 bf16 weights+features (recentered basis keeps the
                 quadratic cancellation-free; end-to-end err ~4e-3 vs the
                 2e-2 budget). Two matmuls (cluster halves) -> E in PSUM
                 [128 parts = 8 clusters x 16 groups, 2*512].
  dens           exp split across engines so neither is the bottleneck:
                 ACT does exact exp on half0 cols, DVE does a Schraudolph
                 bf16 exp (bits = u16(E*128/ln2 + C)) on half1 cols.
  g = C^T dens   sign-combine matmuls with col-group tile_position packing:
                 4 concurrent 16-partition matmuls land 2 super-tiles in
                 one PSUM bank (parts 0:16/32:48/64:80/96:112), so one
                 [128, 1024] PSUM tile holds 4 super-tiles of g.
  g -> DRAM      DMA'd straight from PSUM (no SBUF staging copy), f32.
                 Host adds the two half-partitions, squares, reduces.

DMA dispatches (~0.7us each on the SP queue) are batched; rhs DRAM layout
is [96, NST*FD] bf16 so one DMA loads a run of super-tiles.
"""

import numpy as np

import concourse.bass as bass
import concourse.mybir as mybir
import concourse.tile as tile
from concourse import bacc
from concourse.bass_utils import run_bass_kernel_spmd

N_SAMPLES = 2_000_000
N_CORES = 8
NC_SAMP = N_SAMPLES // N_CORES  # 250_000
K = 16  # clusters
NF = 6  # features
G = 16  # sample groups (one per matmul output column block)
FD = 512  # moving free dim (one PSUM bank of fp32)
SUPER = G * FD  # samples per super-tile = 8192
NST = -(-NC_SAMP // SUPER)  # 31 super-tiles/core
NPAD = NST * SUPER  # 253952 padded samples per core
NPACK = -(-NST // 6)  # g packs of 6 super-tiles (3 bases x 2 banks)
PAD_U = 1.0e6  # pad feature: huge y0^2 --> E << 0 --> dens = 0
# rhs DMA chunk sizes (super-tiles): small first chunks so compute starts
# early, then large chunks to amortize the ~625ns serialized DGE overhead.
RHS_CHUNKS = [2, 2, 4, 6, 8, 9]  # pair-aligned boundaries
assert sum(RHS_CHUNKS) == NST

TWO_PI = 2.0 * np.pi
# bf16-Schraudolph exp constants: bits(bf16 exp(E)) ~= E*128/ln2 + (127*128-C2)
# C2 = 8.0 calibrated end-to-end (cancels the piecewise-linear bias).
SCHRAUD_A2 = float(128.0 / np.log(2.0))
SCHRAUD_B2 = float(127.0 * 128.0 - 8.0)
# pipeline tuning knobs
RHS_BUFS = 3
DA_BUFS = 3
DB_BUFS = 3
E_BUFS = 2
G_BUFS = 2
PIPE_D = 2

_CACHE = {}


def _cluster_params(means, chols, weights):
    """A [K,6] f64 monomial coefficients in a re-centered basis (incl. the
    -0.5 factor and ln|c| const), signs [K], and the center ctr [2].

    Centering at a precision-weighted mean of the cluster means kills the
    catastrophic cancellation that tight clusters (large S^-1) otherwise
    cause in the expanded quadratic, which is what makes bf16 operands
    accurate enough end-to-end."""
    means = np.asarray(means, np.float64)
    chols = np.asarray(chols, np.float64)
    weights = np.asarray(weights, np.float64)
    L = np.tril(chols)
    S = L @ np.swapaxes(L, 1, 2)
    P = np.linalg.inv(S)
    detS = np.linalg.det(S)
    c = weights / np.sqrt(TWO_PI**2 * detS)
    signs = np.where(c >= 0, 1.0, -1.0)
    logc = np.log(np.abs(c))
    pw = np.abs(P).sum(axis=(1, 2))
    ctr = (means * pw[:, None]).sum(0) / pw.sum()
    m = means - ctr[None, :]
    Pm = np.einsum("kij,kj->ki", P, m)
    mPm = np.einsum("ki,ki->k", m, Pm)
    A = np.stack(
        [
            -0.5 * P[:, 0, 0],
            -P[:, 0, 1],
            -0.5 * P[:, 1, 1],
            Pm[:, 0],
            Pm[:, 1],
            -0.5 * mPm + logc,
        ],
        axis=1,
    )
    return A, signs, ctr


def _z_term(means, chols, weights):
    means = np.asarray(means, np.float64)
    chols = np.asarray(chols, np.float64)
    weights = np.asarray(weights, np.float64)
    L = np.tril(chols)
    S = L @ np.swapaxes(L, 1, 2)
    Ssum = S[:, None] + S[None, :]
    mdiff = means[:, None, :] - means[None, :, :]
    m2 = np.einsum("abi,abij,abj->ab", mdiff, np.linalg.inv(Ssum), mdiff)
    Zij = np.exp(-0.5 * m2) / np.sqrt(TWO_PI**2 * np.linalg.det(Ssum))
    return float(np.einsum("i,j,ij->", weights, weights, Zij))


def _build_rhs(X, ctr):
    """X [2M,2] f32 -> per-core rhs [N_CORES, 96, NST*FD] bf16, where
    rhs[c, g*NF+f, st*FD + t] = feat_f of sample
    n = c*NC_SAMP + st*SUPER + g*FD + t  (pad samples give dens == 0)."""
    import ml_dtypes

    X = np.asarray(X, np.float32)
    feats = np.empty((N_CORES, NPAD, NF), np.float32)
    x0 = (X[:, 0] - np.float32(ctr[0])).reshape(N_CORES, NC_SAMP)
    x1 = (X[:, 1] - np.float32(ctr[1])).reshape(N_CORES, NC_SAMP)
    feats[:, :NC_SAMP, 0] = x0 * x0
    feats[:, :NC_SAMP, 1] = x0 * x1
    feats[:, :NC_SAMP, 2] = x1 * x1
    feats[:, :NC_SAMP, 3] = x0
    feats[:, :NC_SAMP, 4] = x1
    feats[:, :NC_SAMP, 5] = 1.0
    feats[:, NC_SAMP:, :] = 0.0
    feats[:, NC_SAMP:, 0] = PAD_U
    feats[:, NC_SAMP:, 5] = 1.0
    # [C, NST, G, FD, NF] -> [C, G, NF, NST, FD] -> [C, 96, NST*FD]
    r = feats.reshape(N_CORES, NST, G, FD, NF).transpose(0, 2, 4, 1, 3)
    return np.ascontiguousarray(r).reshape(N_CORES, G * NF, NST * FD).astype(
        ml_dtypes.bfloat16
    )


def _build_weights(A32, signs):
    """w1 [G*NF, 256] bf16 block coefficient mats (cluster halves);
    cm [2, 128, G] bf16 sign-combine mats. Out partition m = k_local*G+g."""
    import ml_dtypes

    w1 = np.zeros((G * NF, 2 * 128), np.float32)
    cm = np.zeros((128, 2 * G), np.float32)
    for half in (0, 1):
        for kl in range(8):
            k = half * 8 + kl
            for g in range(G):
                w1[g * NF : (g + 1) * NF, half * 128 + kl * G + g] = A32[k]
                cm[kl * G + g, half * G + g] = signs[k]
    return w1.astype(ml_dtypes.bfloat16), cm.astype(ml_dtypes.bfloat16)


def _build_bass():
    nc = bacc.Bacc("TRN2", target_bir_lowering=False, debug=False)
    f32 = mybir.dt.float32
    bf16 = mybir.dt.bfloat16
    u16 = mybir.dt.uint16
    rhs_d = nc.dram_tensor("rhs", [G * NF, NST * FD], bf16, kind="ExternalInput")
    w1_d = nc.dram_tensor("w1", [G * NF, 2 * 128], bf16, kind="ExternalInput")
    cm_d = nc.dram_tensor("cm", [128, 2 * G], bf16, kind="ExternalInput")
    gout_d = nc.dram_tensor("gout", [128, 2 * NPACK], f32, kind="ExternalOutput")

    with tile.TileContext(nc) as tc:
        with (
            tc.tile_pool(name="const", bufs=1) as cpool,
            tc.tile_pool(name="rhs", bufs=RHS_BUFS) as rpool,
            tc.tile_pool(name="densa", bufs=DA_BUFS) as dapool,
            tc.tile_pool(name="densb", bufs=DB_BUFS) as dbpool,
            tc.tile_pool(name="pea", bufs=E_BUFS, space="PSUM") as eapool,
            tc.tile_pool(name="peb", bufs=E_BUFS, space="PSUM") as ebpool,
            tc.tile_pool(name="pg", bufs=G_BUFS, space="PSUM") as gpool,
        ):
            # PE warm-up on a memset tile: keeps the HAM clock-gate from
            # throttling the first real matmuls while the w1/rhs DMAs are
            # still in flight.
            wz = cpool.tile([128, 128], bf16)
            nc.gpsimd.memset(wz[:], 0)
            warm = gpool.tile([128, 128], f32, tag="g")
            for _ in range(10):
                nc.tensor.matmul(
                    warm[:], wz[:], wz[:], start=True, stop=True
                )

            w1 = cpool.tile([G * NF, 2 * 128], bf16)
            cm = cpool.tile([128, 2 * G], bf16)
            acc = cpool.tile([128, 2 * NPACK], f32)

            rhs_views = {}  # st -> AP slice of its chunk tile
            lo = 0
            for ci, sz in enumerate(RHS_CHUNKS):
                hi = lo + sz
                rt = rpool.tile([G * NF, max(RHS_CHUNKS) * FD], bf16, tag="rhs")
                nc.sync.dma_start(rt[:, : sz * FD], rhs_d[:, lo * FD : hi * FD])
                if ci == 1:
                    nc.sync.dma_start(w1[:], w1_d[:])
                elif ci == 3:
                    nc.sync.dma_start(cm[:], cm_d[:])
                for st in range(lo, hi):
                    rhs_views[st] = (rt, (st - lo) * FD)
                lo = hi

            # Software pipeline at super-tile-PAIR granularity: rhs chunks
            # are pair-aligned so mm1a / mm2 run as single 1024-col matmuls
            # per pair (half the PE instruction + LDWEIGHTS count). The
            # sign-combine stage is delayed by D pairs so PE's in-order
            # stream never blocks on the exp engines.
            D = PIPE_D
            NPAIR = -(-NST // 2)  # 16, last pair is st30 alone
            dens_ring = [None] * NPAIR
            g_hold = [None]

            def emit_front(p):
                st0 = 2 * p
                full = st0 + 1 < NST
                w = 2 * FD if full else FD
                # E tiles are separate per cluster-half so the ACT and DVE
                # exp chains stay independent (a shared tile makes the
                # scheduler serialize DVE behind ACT).
                ea = eapool.tile([128, 2 * FD], f32, tag="ea", name=f"ea{p}")
                da = dapool.tile([128, 2 * FD], bf16, tag="densa", name=f"da{p}")
                db = dbpool.tile([128, 2 * FD], u16, tag="densb", name=f"db{p}")
                for h in range(2 if full else 1):
                    rt, lo = rhs_views[st0 + h]
                    rhs = rt[:, lo : lo + FD]
                    nc.tensor.matmul(
                        ea[:, h * FD : (h + 1) * FD], w1[:, 0:128], rhs,
                        start=True, stop=True,
                    )
                    ebh = ebpool.tile([128, FD], f32, tag="eb", name=f"eb{p}_{h}")
                    nc.tensor.matmul(
                        ebh[:], w1[:, 128:256], rhs, start=True, stop=True
                    )
                    # DVE half: Schraudolph bf16 exp (the f32->u16 convert
                    # saturates negatives to 0 == bf16 +0.0, so the
                    # underflow band needs no clamp; ~1% sawtooth error
                    # washes out over the 2M-sample reduction).
                    nc.vector.tensor_scalar(
                        db[:, h * FD : (h + 1) * FD],
                        ebh[:],
                        SCHRAUD_A2,
                        SCHRAUD_B2,
                        op0=mybir.AluOpType.mult,
                        op1=mybir.AluOpType.add,
                    )
                # one exact exp on ACT for the pair's half0 energies
                nc.scalar.activation(
                    da[:, 0:w], ea[:, 0:w], mybir.ActivationFunctionType.Exp
                )
                dens_ring[p] = (da, db, w)

            def emit_back(p):
                da, db, w = dens_ring[p]
                dens_ring[p] = None
                # 3 pairs per g pack at partition bases 0/32/64 (base 96 =
                # quadrant 3 is a HW no-go); pair halves land in two
                # separate single-bank g tiles so each can be squared (and
                # its buffer recycled) as soon as its 3 slots are full.
                # h0/h1 cluster-halves accumulate in PSUM per slot.
                pk, sp = p // 3, p % 3
                last = p == NPAIR - 1
                if sp == 0:
                    nh = 2 if (not last or w == 2 * FD) else 1
                    g_hold[0] = [
                        gpool.tile([128, FD], f32, tag="g", name=f"gt{pk}_{h}")
                        for h in range(nh)
                    ]
                gs = g_hold[0]
                pbase = 32 * sp
                for h in range(w // FD):
                    region = gs[h][pbase : pbase + G, :]
                    sl = slice(h * FD, (h + 1) * FD)
                    nc.tensor.matmul(
                        region, cm[:, 0:G], da[:, sl],
                        start=True, stop=False, skip_group_check=True,
                    )
                    nc.tensor.matmul(
                        region, cm[:, G : 2 * G], db[:, sl].bitcast(bf16),
                        start=False, stop=True, skip_group_check=True,
                    )
                    if sp == 2:
                        # half-pack complete: sum-of-squares, in place
                        nc.scalar.activation(
                            gs[h][:],
                            gs[h][:],
                            mybir.ActivationFunctionType.Square,
                            accum_out=acc[:, 2 * pk + h : 2 * pk + h + 1],
                        )
                if last and sp != 2:
                    # partial pack: per-pair-region squares
                    for q in range(3 * pk, NPAIR):
                        qb = 32 * (q % 3)
                        for h in range((2 * FD if 2 * q + 1 < NST else FD) // FD):
                            rg = gs[h][qb : qb + G, :]
                            nc.scalar.activation(
                                rg,
                                rg,
                                mybir.ActivationFunctionType.Square,
                                accum_out=acc[qb : qb + G, 2 * pk + h : 2 * pk + h + 1],
                            )

            for p in range(NPAIR + D):
                if p < NPAIR:
                    emit_front(p)
                if p >= D:
                    emit_back(p - D)

            nc.sync.dma_start(gout_d[:], acc[:])

    nc.compile()
    return nc


def _get_bass():
    if "nc" not in _CACHE:
        _CACHE["nc"] = _build_bass()
    return _CACHE["nc"]


def kernel(X, means, chols, weights, it=None, **_unused):
    X = np.ascontiguousarray(np.asarray(X, np.float32))
    assert X.shape == (N_SAMPLES, 2), X.shape

    A, signs, ctr = _cluster_params(means, chols, weights)
    A32 = A.astype(np.float32)
    z = _z_term(means, chols, weights)

    w1, cm = _build_weights(A32, signs)
    rhs = _build_rhs(X, ctr)

    nc = _get_bass()
    in_maps = [{"rhs": rhs[c], "w1": w1, "cm": cm} for c in range(N_CORES)]
    res = run_bass_kernel_spmd(nc, in_maps, core_ids=list(range(N_CORES)))

    total = 0.0
    for r in res.results:
        go = r["gout"].astype(np.float64)  # [128, 2*NPACK] half-pack sum(g^2)
        # acc[32*(p%3) + 0:16, 2*(p//3) + h] holds the sum-of-squares for
        # super-tile 2p+h; other rows are garbage.
        npair = -(-NST // 2)
        for p in range(npair):
            pb = 32 * (p % 3)
            for h in range(2 if 2 * p + 1 < NST else 1):
                total += float(go[pb : pb + G, 2 * (p // 3) + h].sum())

    out = -(np.log(total) - np.log(z)) / N_SAMPLES
    return np.float32(out)


if __name__ == "__main__":
    rng = np.random.default_rng(0)
    X = rng.standard_normal((N_SAMPLES, 2), dtype=np.float32)
    scale = 2.0 * (1.0 + rng.standard_normal((K, 1, 1), dtype=np.float32))
    chols = scale * np.ones((2, 2), np.float32)[None] + 0.5 * np.eye(2, dtype=np.float32)[None]
    means = rng.standard_normal((K, 2), dtype=np.float32)
    weights = rng.standard_normal(K, dtype=np.float32)
    print(kernel(X, means, chols, weights, 1))


# revision 43
# speedup vs baseline: 1.0320x; 1.0320x over previous
"""Trainium2 Bass kernel for nn_NMSquaredGaussianMixture.

Math: output = -(log(sum_n g_n^2) - log z) / N
  g_n = sum_k c_k * exp(E_k(x_n)),  c_k = w_k / sqrt((2pi)^2 det S_k)
  E_k(x) = -0.5 (x-mu_k)^T S_k^{-1} (x-mu_k)
  z     = sum_ij w_i w_j N(mu_i - mu_j; 0, S_i + S_j)   (tiny, host-side)

Device pipeline (per core, data-parallel over samples), per super-tile of
8192 samples (16 groups x 512):
  E = W1 @ F     features F = [y0^2, y0*y1, y1^2, y0, y1, 1], y = x - ctr
                 bf16 weights+features (recentered basis keeps the
                 quadratic cancellation-free; end-to-end err ~4e-3 vs the
                 2e-2 budget). Two matmuls (cluster halves) -> E in PSUM
                 [128 parts = 8 clusters x 16 groups, 2*512].
  dens           exp split across engines so neither is the bottleneck:
                 ACT does exact exp on half0 cols, DVE does a Schraudolph
                 bf16 exp (bits = u16(E*128/ln2 + C)) on half1 cols.
  g = C^T dens   sign-combine matmuls with col-group tile_position packing:
                 4 concurrent 16-partition matmuls land 2 super-tiles in
                 one PSUM bank (parts 0:16/32:48/64:80/96:112), so one
                 [128, 1024] PSUM tile holds 4 super-tiles of g.
  g -> DRAM      DMA'd straight from PSUM (no SBUF staging copy), f32.
                 Host adds the two half-partitions, squares, reduces.

DMA dispatches (~0.7us each on the SP queue) are batched; rhs DRAM layout
is [96, NST*FD] bf16 so one DMA loads a run of super-tiles.
"""

import numpy as np

import concourse.bass as bass
import concourse.mybir as mybir
import concourse.tile as tile
from concourse import bacc
from concourse.bass_utils import run_bass_kernel_spmd

N_SAMPLES = 2_000_000
N_CORES = 8
NC_SAMP = N_SAMPLES // N_CORES  # 250_000
K = 16  # clusters
NF = 6  # features
G = 16  # sample groups (one per matmul output column block)
FD = 512  # moving free dim (one PSUM bank of fp32)
SUPER = G * FD  # samples per super-tile = 8192
NST = -(-NC_SAMP // SUPER)  # 31 super-tiles/core
NPAD = NST * SUPER  # 253952 padded samples per core
NPACK = -(-NST // 6)  # g packs of 6 super-tiles (3 bases x 2 banks)
PAD_U = 1.0e6  # pad feature: huge y0^2 --> E << 0 --> dens = 0
# rhs DMA chunk sizes (super-tiles): small first chunks so compute starts
# early, then large chunks to amortize the ~625ns serialized DGE overhead.
RHS_CHUNKS = [2, 2, 2, 2, 3, 4, 4, 6, 6]  # pair-aligned boundaries
assert sum(RHS_CHUNKS) == NST

TWO_PI = 2.0 * np.pi
# bf16-Schraudolph exp constants: bits(bf16 exp(E)) ~= E*128/ln2 + (127*128-C2)
# C2 = 8.0 calibrated end-to-end (cancels the piecewise-linear bias).
SCHRAUD_A2 = float(128.0 / np.log(2.0))
SCHRAUD_B2 = float(127.0 * 128.0 - 8.0)
# pipeline tuning knobs
RHS_BUFS = 3
DA_BUFS = 3
DB_BUFS = 3
E_BUFS = 2
G_BUFS = 2
PIPE_D = 2

_CACHE = {}


def _cluster_params(means, chols, weights):
    """A [K,6] f64 monomial coefficients in a re-centered basis (incl. the
    -0.5 factor and ln|c| const), signs [K], and the center ctr [2].

    Centering at a precision-weighted mean of the cluster means kills the
    catastrophic cancellation that tight clusters (large S^-1) otherwise
    cause in the expanded quadratic, which is what makes bf16 operands
    accurate enough end-to-end."""
    means = np.asarray(means, np.float64)
    chols = np.asarray(chols, np.float64)
    weights = np.asarray(weights, np.float64)
    L = np.tril(chols)
    S = L @ np.swapaxes(L, 1, 2)
    P = np.linalg.inv(S)
    detS = np.linalg.det(S)
    c = weights / np.sqrt(TWO_PI**2 * detS)
    signs = np.where(c >= 0, 1.0, -1.0)
    logc = np.log(np.abs(c))
    pw = np.abs(P).sum(axis=(1, 2))
    ctr = (means * pw[:, None]).sum(0) / pw.sum()
    m = means - ctr[None, :]
    Pm = np.einsum("kij,kj->ki", P, m)
    mPm = np.einsum("ki,ki->k", m, Pm)
    A = np.stack(
        [
            -0.5 * P[:, 0, 0],
            -P[:, 0, 1],
            -0.5 * P[:, 1, 1],
            Pm[:, 0],
            Pm[:, 1],
            -0.5 * mPm + logc,
        ],
        axis=1,
    )
    return A, signs, ctr


def _z_term(means, chols, weights):
    means = np.asarray(means, np.float64)
    chols = np.asarray(chols, np.float64)
    weights = np.asarray(weights, np.float64)
    L = np.tril(chols)
    S = L @ np.swapaxes(L, 1, 2)
    Ssum = S[:, None] + S[None, :]
    mdiff = means[:, None, :] - means[None, :, :]
    m2 = np.einsum("abi,abij,abj->ab", mdiff, np.linalg.inv(Ssum), mdiff)
    Zij = np.exp(-0.5 * m2) / np.sqrt(TWO_PI**2 * np.linalg.det(Ssum))
    return float(np.einsum("i,j,ij->", weights, weights, Zij))


def _build_rhs(X, ctr):
    """X [2M,2] f32 -> per-core rhs [N_CORES, 96, NST*FD] bf16, where
    rhs[c, g*NF+f, st*FD + t] = feat_f of sample
    n = c*NC_SAMP + st*SUPER + g*FD + t  (pad samples give dens == 0)."""
    import ml_dtypes

    X = np.asarray(X, np.float32)
    feats = np.empty((N_CORES, NPAD, NF), np.float32)
    x0 = (X[:, 0] - np.float32(ctr[0])).reshape(N_CORES, NC_SAMP)
    x1 = (X[:, 1] - np.float32(ctr[1])).reshape(N_CORES, NC_SAMP)
    feats[:, :NC_SAMP, 0] = x0 * x0
    feats[:, :NC_SAMP, 1] = x0 * x1
    feats[:, :NC_SAMP, 2] = x1 * x1
    feats[:, :NC_SAMP, 3] = x0
    feats[:, :NC_SAMP, 4] = x1
    feats[:, :NC_SAMP, 5] = 1.0
    feats[:, NC_SAMP:, :] = 0.0
    feats[:, NC_SAMP:, 0] = PAD_U
    feats[:, NC_SAMP:, 5] = 1.0
    # [C, NST, G, FD, NF] -> [C, G, NF, NST, FD] -> [C, 96, NST*FD]
    r = feats.reshape(N_CORES, NST, G, FD, NF).transpose(0, 2, 4, 1, 3)
    return np.ascontiguousarray(r).reshape(N_CORES, G * NF, NST * FD).astype(
        ml_dtypes.bfloat16
    )


def _build_weights(A32, signs):
    """w1 [G*NF, 256] bf16 block coefficient mats (cluster halves);
    cm [2, 128, G] bf16 sign-combine mats. Out partition m = k_local*G+g."""
    import ml_dtypes

    w1 = np.zeros((G * NF, 2 * 128), np.float32)
    cm = np.zeros((128, 2 * G), np.float32)
    for half in (0, 1):
        for kl in range(8):
            k = half * 8 + kl
            for g in range(G):
                w1[g * NF : (g + 1) * NF, half * 128 + kl * G + g] = A32[k]
                cm[kl * G + g, half * G + g] = signs[k]
    return w1.astype(ml_dtypes.bfloat16), cm.astype(ml_dtypes.bfloat16)


def _build_bass():
    nc = bacc.Bacc("TRN2", target_bir_lowering=False, debug=False)
    f32 = mybir.dt.float32
    bf16 = mybir.dt.bfloat16
    u16 = mybir.dt.uint16
    rhs_d = nc.dram_tensor("rhs", [G * NF, NST * FD], bf16, kind="ExternalInput")
    w1_d = nc.dram_tensor("w1", [G * NF, 2 * 128], bf16, kind="ExternalInput")
    cm_d = nc.dram_tensor("cm", [128, 2 * G], bf16, kind="ExternalInput")
    gout_d = nc.dram_tensor("gout", [128, 2 * NPACK], f32, kind="ExternalOutput")

    with tile.TileContext(nc) as tc:
        with (
            tc.tile_pool(name="const", bufs=1) as cpool,
            tc.tile_pool(name="rhs", bufs=RHS_BUFS) as rpool,
            tc.tile_pool(name="densa", bufs=DA_BUFS) as dapool,
            tc.tile_pool(name="densb", bufs=DB_BUFS) as dbpool,
            tc.tile_pool(name="pea", bufs=E_BUFS, space="PSUM") as eapool,
            tc.tile_pool(name="peb", bufs=E_BUFS, space="PSUM") as ebpool,
            tc.tile_pool(name="pg", bufs=G_BUFS, space="PSUM") as gpool,
        ):
            # PE warm-up on a memset tile: keeps the HAM clock-gate from
            # throttling the first real matmuls while the w1/rhs DMAs are
            # still in flight.
            wz = cpool.tile([128, 128], bf16)
            nc.gpsimd.memset(wz[:], 0)
            warm = gpool.tile([128, 128], f32, tag="g")
            for _ in range(10):
                nc.tensor.matmul(
                    warm[:], wz[:], wz[:], start=True, stop=True
                )

            w1 = cpool.tile([G * NF, 2 * 128], bf16)
            cm = cpool.tile([128, 2 * G], bf16)
            acc = cpool.tile([128, 2 * NPACK], f32)

            # rhs loads alternate between the two HWDGE queues (SP + ACT)
            # so dispatch + descriptor-gen for early chunks parallelize;
            # ACT is idle until the first exp anyway.
            nc.scalar.dma_start(w1[:], w1_d[:])
            rhs_views = {}  # st -> (chunk tile, col offset)
            lo = 0
            for ci, sz in enumerate(RHS_CHUNKS):
                hi = lo + sz
                rt = rpool.tile([G * NF, max(RHS_CHUNKS) * FD], bf16, tag="rhs")
                eng = nc.sync if ci % 2 == 0 else nc.scalar
                eng.dma_start(rt[:, : sz * FD], rhs_d[:, lo * FD : hi * FD])
                if ci == 2:
                    nc.sync.dma_start(cm[:], cm_d[:])
                for st in range(lo, hi):
                    rhs_views[st] = (rt, (st - lo) * FD)
                lo = hi

            # Software pipeline at super-tile-PAIR granularity: rhs chunks
            # are pair-aligned so mm1a / mm2 run as single 1024-col matmuls
            # per pair (half the PE instruction + LDWEIGHTS count). The
            # sign-combine stage is delayed by D pairs so PE's in-order
            # stream never blocks on the exp engines.
            D = PIPE_D
            NPAIR = -(-NST // 2)  # 16, last pair is st30 alone
            dens_ring = [None] * NPAIR
            g_hold = [None]

            def emit_front(p):
                st0 = 2 * p
                full = st0 + 1 < NST
                w = 2 * FD if full else FD
                # E tiles are separate per cluster-half so the ACT and DVE
                # exp chains stay independent (a shared tile makes the
                # scheduler serialize DVE behind ACT).
                ea = eapool.tile([128, 2 * FD], f32, tag="ea", name=f"ea{p}")
                da = dapool.tile([128, 2 * FD], bf16, tag="densa", name=f"da{p}")
                db = dbpool.tile([128, 2 * FD], u16, tag="densb", name=f"db{p}")
                for h in range(2 if full else 1):
                    rt, lo = rhs_views[st0 + h]
                    rhs = rt[:, lo : lo + FD]
                    nc.tensor.matmul(
                        ea[:, h * FD : (h + 1) * FD], w1[:, 0:128], rhs,
                        start=True, stop=True,
                    )
                    ebh = ebpool.tile([128, FD], f32, tag="eb", name=f"eb{p}_{h}")
                    nc.tensor.matmul(
                        ebh[:], w1[:, 128:256], rhs, start=True, stop=True
                    )
                    # DVE half: Schraudolph bf16 exp (the f32->u16 convert
                    # saturates negatives to 0 == bf16 +0.0, so the
                    # underflow band needs no clamp; ~1% sawtooth error
                    # washes out over the 2M-sample reduction).
                    nc.vector.tensor_scalar(
                        db[:, h * FD : (h + 1) * FD],
                        ebh[:],
                        SCHRAUD_A2,
                        SCHRAUD_B2,
                        op0=mybir.AluOpType.mult,
                        op1=mybir.AluOpType.add,
                    )
                # one exact exp on ACT for the pair's half0 energies
                nc.scalar.activation(
                    da[:, 0:w], ea[:, 0:w], mybir.ActivationFunctionType.Exp
                )
                dens_ring[p] = (da, db, w)

            def emit_back(p):
                da, db, w = dens_ring[p]
                dens_ring[p] = None
                # 3 pairs per g pack at partition bases 0/32/64 (base 96 =
                # quadrant 3 is a HW no-go); pair halves land in two
                # separate single-bank g tiles so each can be squared (and
                # its buffer recycled) as soon as its 3 slots are full.
                # h0/h1 cluster-halves accumulate in PSUM per slot.
                pk, sp = p // 3, p % 3
                last = p == NPAIR - 1
                if sp == 0:
                    nh = 2 if (not last or w == 2 * FD) else 1
                    g_hold[0] = [
                        gpool.tile([128, FD], f32, tag="g", name=f"gt{pk}_{h}")
                        for h in range(nh)
                    ]
                gs = g_hold[0]
                pbase = 32 * sp
                for h in range(w // FD):
                    region = gs[h][pbase : pbase + G, :]
                    sl = slice(h * FD, (h + 1) * FD)
                    nc.tensor.matmul(
                        region, cm[:, 0:G], da[:, sl],
                        start=True, stop=False, skip_group_check=True,
                    )
                    nc.tensor.matmul(
                        region, cm[:, G : 2 * G], db[:, sl].bitcast(bf16),
                        start=False, stop=True, skip_group_check=True,
                    )
                    if sp == 2:
                        # half-pack complete: sum-of-squares, in place
                        nc.scalar.activation(
                            gs[h][:],
                            gs[h][:],
                            mybir.ActivationFunctionType.Square,
                            accum_out=acc[:, 2 * pk + h : 2 * pk + h + 1],
                        )
                if last and sp != 2:
                    # partial pack: per-pair-region squares
                    for q in range(3 * pk, NPAIR):
                        qb = 32 * (q % 3)
                        for h in range((2 * FD if 2 * q + 1 < NST else FD) // FD):
                            rg = gs[h][qb : qb + G, :]
                            nc.scalar.activation(
                                rg,
                                rg,
                                mybir.ActivationFunctionType.Square,
                                accum_out=acc[qb : qb + G, 2 * pk + h : 2 * pk + h + 1],
                            )

            for p in range(NPAIR + D):
                if p < NPAIR:
                    emit_front(p)
                if p >= D:
                    emit_back(p - D)

            nc.sync.dma_start(gout_d[:], acc[:])

    nc.compile()
    return nc


def _get_bass():
    if "nc" not in _CACHE:
        _CACHE["nc"] = _build_bass()
    return _CACHE["nc"]


def kernel(X, means, chols, weights, it=None, **_unused):
    X = np.ascontiguousarray(np.asarray(X, np.float32))
    assert X.shape == (N_SAMPLES, 2), X.shape

    A, signs, ctr = _cluster_params(means, chols, weights)
    A32 = A.astype(np.float32)
    z = _z_term(means, chols, weights)

    w1, cm = _build_weights(A32, signs)
    rhs = _build_rhs(X, ctr)

    nc = _get_bass()
    in_maps = [{"rhs": rhs[c], "w1": w1, "cm": cm} for c in range(N_CORES)]
    res = run_bass_kernel_spmd(nc, in_maps, core_ids=list(range(N_CORES)))

    total = 0.0
    for r in res.results:
        go = r["gout"].astype(np.float64)  # [128, 2*NPACK] half-pack sum(g^2)
        # acc[32*(p%3) + 0:16, 2*(p//3) + h] holds the sum-of-squares for
        # super-tile 2p+h; other rows are garbage.
        npair = -(-NST // 2)
        for p in range(npair):
            pb = 32 * (p % 3)
            for h in range(2 if 2 * p + 1 < NST else 1):
                total += float(go[pb : pb + G, 2 * (p // 3) + h].sum())

    out = -(np.log(total) - np.log(z)) / N_SAMPLES
    return np.float32(out)


if __name__ == "__main__":
    rng = np.random.default_rng(0)
    X = rng.standard_normal((N_SAMPLES, 2), dtype=np.float32)
    scale = 2.0 * (1.0 + rng.standard_normal((K, 1, 1), dtype=np.float32))
    chols = scale * np.ones((2, 2), np.float32)[None] + 0.5 * np.eye(2, dtype=np.float32)[None]
    means = rng.standard_normal((K, 2), dtype=np.float32)
    weights = rng.standard_normal(K, dtype=np.float32)
    print(kernel(X, means, chols, weights, 1))
